# revision 25
# baseline (speedup 1.0000x reference)
"""Trainium2 kernel for nn_ButterworthFilter: 4th-order Butterworth lowpass
(scipy.signal.butter(4, 0.5) equivalent) applied as an IIR filter along time
for x of shape [256, 65536, 1], zero initial state per batch row.

Strategy
--------
The IIR impulse response decays below 1e-15 within 128 taps (max pole radius
0.7577), so the filter is numerically exactly a 128-tap causal FIR:

    y[t] = sum_{k=0}^{127} h[k] x[t-k]

Blocking time into 128-sample blocks, with X_cols[m, b] = x[128 b + m]:

    y[128 b + i] = sum_m W_A[m, i] X_cols[m, b] + sum_m W_B[m, i] X_cols[m, b-1]

with W_A[m, i] = h[i - m] (lower-triangular Toeplitz) and
W_B[m, i] = h[128 + i - m] (strictly upper-triangular). On the PE array this
is two accumulating matmuls per 512-block group with the contraction (m) on
partitions.

Sharding: pure data-parallel, 32 batch rows per core across 8 cores.

The natural->blocked layout change (and its inverse for y) is done on the
HOST, so the device only streams [128, 512] tiles: DMA in -> 2 matmuls ->
PSUM -> copy -> DMA out. The kernel is DMA-bound; mode "fp16" halves the
HBM traffic (absmax error ~5e-4 of output scale vs the fp32 reference;
mode "fp32r" keeps fp32 I/O at ~2.4e-4; mode "fp32" is ~1e-6 but slower).
"""
import os

import numpy as np

N_CORES = 8
B = 256
T = 65536
ROWS = B // N_CORES  # 32 batch rows per core
NBLK = T // 128  # 512 blocks of 128 samples per row
ORDER = 4

# per-chunk row counts: small chunks first (compute starts early) and last
# (short tail), big in the middle (few DMA triggers)
CHUNKS = [int(c) for c in os.environ.get("BUTTER_CHUNKS", "2,2,4,8,8,4,2,1,1").split(",")]
assert sum(CHUNKS) == ROWS
# "fp16" | "fp32r" | "fp32"
MODE = os.environ.get("BUTTER_MODE", "fp16")


def _design_fir(n_taps: int = 128) -> np.ndarray:
    """Butterworth(4, Wn=0.5) digital filter -> first n_taps of the impulse
    response, in float64. Same math as scipy.signal.butter(4, 0.5, 'low')."""
    fs2 = 4.0
    warped = fs2 * np.tan(np.pi * 0.5 / 4.0)
    k = np.arange(1, ORDER + 1)
    p = warped * np.exp(1j * np.pi * (2 * k + ORDER - 1) / (2 * ORDER))
    pd = (fs2 + p) / (fs2 - p)
    kd = (warped**ORDER) / np.real(np.prod(fs2 - p))
    b = np.real(kd * np.poly(-np.ones(ORDER)))
    a = np.real(np.poly(pd))

    h = np.zeros(n_taps)
    z = np.zeros(ORDER)
    for t in range(n_taps):
        xt = 1.0 if t == 0 else 0.0
        y = b[0] * xt + z[0]
        z = np.concatenate([z[1:], [0.0]]) + b[1:] * xt - a[1:] * y
        h[t] = y
    return h


def _toeplitz_weights() -> tuple[np.ndarray, np.ndarray]:
    h = _design_fir(128)
    idx = np.arange(128)
    d = idx[None, :] - idx[:, None]  # i - m
    w_a = np.where(d >= 0, h[np.clip(d, 0, 127)], 0.0)
    d2 = 128 + idx[None, :] - idx[:, None]
    w_b = np.where((d2 >= 1) & (d2 <= 127), h[np.clip(d2, 0, 127)], 0.0)
    return w_a.astype(np.float32), w_b.astype(np.float32)


_NC_CACHE = None

_IO_NP = {"fp16": np.float16, "fp32r": np.float32, "fp32": np.float32}


def _build_bass():
    """Build (and cache) the compiled per-core Bass program."""
    global _NC_CACHE
    if _NC_CACHE is not None:
        return _NC_CACHE

    import concourse.tile as tile
    from concourse import bacc, mybir

    w_a, w_b = _toeplitz_weights()

    if MODE == "fp16":
        io_dt = mm_dt = mybir.dt.float16
        w_a, w_b = w_a.astype(np.float16), w_b.astype(np.float16)
    elif MODE == "fp32r":
        io_dt = mm_dt = mybir.dt.float32r
    else:
        io_dt = mm_dt = mybir.dt.float32

    nc = bacc.Bacc("TRN2", target_bir_lowering=False, debug=False)
    # host-packed input, partition-major so each partition's DMA run is
    # CHUNK_ROWS*513 contiguous elements: [128, ROWS, 513];
    # [m, r, 0] = 0 (the b=-1 column), [m, r, 1 + b] = x[row r, 128 b + m]
    xb = nc.dram_tensor("xb", [128, ROWS, NBLK + 1], io_dt, kind="ExternalInput").ap()
    # output, partition-major: [128, ROWS, 512]; [i, r, b] = y[row r, 128 b + i]
    yb = nc.dram_tensor("yb", [128, ROWS, NBLK], io_dt, kind="ExternalOutput").ap()
    wa_dram = nc.inline_tensor(w_a, name="wa_const")
    wb_dram = nc.inline_tensor(w_b, name="wb_const")

    n_chunks = len(CHUNKS)

    with tile.TileContext(nc) as tc:
        with (
            tc.tile_pool(name="wpool", bufs=1) as wpool,
            tc.tile_pool(name="inp", bufs=1) as inp,
            tc.tile_pool(name="outp", bufs=1) as outp,
            tc.tile_pool(name="psum", bufs=8, space="PSUM") as psum_pool,
        ):
            wa_sb = wpool.tile([128, 128], mm_dt, tag="wa")
            nc.gpsimd.dma_start(wa_sb[:], wa_dram.ap().bitcast(mm_dt))
            wb_sb = wpool.tile([128, 128], mm_dt, tag="wb")
            nc.gpsimd.dma_start(wb_sb[:], wb_dram.ap().bitcast(mm_dt))

            # PE warm-up: ~3.4us of dummy matmuls during the DMA ramp releases
            # the HAM clock gate (1.2 -> 2.4 GHz) before the real matmuls start
            scratch = wpool.tile([128, NBLK], io_dt, tag="scratch")
            nc.vector.memset(scratch[:], 0.0)
            ps_warm = psum_pool.tile([128, NBLK], mybir.dt.float32, tag="ps")
            for _ in range(16):
                nc.tensor.matmul(
                    ps_warm[:],
                    scratch[:, 0:128],
                    scratch[:],
                    start=True,
                    stop=True,
                )

            half = NBLK // 2
            r0 = 0
            for c, crows in enumerate(CHUNKS):
                in_t = inp.tile([128, crows, NBLK + 1], io_dt, tag=f"in{c}")
                # alternate the two HWDGE rings so in-DMA triggers ramp 2x faster
                in_eng = nc.sync if c % 2 == 0 else nc.scalar
                in_eng.dma_start(in_t[:], xb[:, r0 : r0 + crows, :])
                out_t = outp.tile([128, crows, NBLK], io_dt, tag=f"out{c}")
                # group all W_A matmuls then all W_B matmuls so consecutive
                # matmuls share the stationary operand (amortize LDWEIGHTS)
                pss = []
                for r in range(crows):
                    ps = psum_pool.tile([128, NBLK], mybir.dt.float32, tag="ps")
                    pss.append(ps)
                    nc.tensor.matmul(
                        ps[:],
                        wa_sb[:],
                        in_t[:, r, 1 : NBLK + 1],
                        start=True,
                        stop=False,
                    )
                for r in range(crows):
                    nc.tensor.matmul(
                        pss[r][:],
                        wb_sb[:],
                        in_t[:, r, 0:NBLK],
                        start=False,
                        stop=True,
                    )
                for r in range(crows):
                    # split the PSUM->SBUF cast copy across DVE and ACT
                    nc.vector.tensor_copy(out_t[:, r, 0:half], pss[r][:, 0:half])
                    nc.scalar.copy(out_t[:, r, half:NBLK], pss[r][:, half:NBLK])
                nc.scalar.dma_start(yb[:, r0 : r0 + crows, :], out_t[:])
                r0 += crows

    nc.compile()
    _NC_CACHE = nc
    return nc


def _pack_core(x_core: np.ndarray) -> np.ndarray:
    """[ROWS, T] float32 -> [128, ROWS, NBLK+1] with a leading zero column."""
    np_dt = _IO_NP[MODE]
    xc = np.zeros((128, ROWS, NBLK + 1), dtype=np_dt)
    # x[row, 128 b + m] -> [m, row, 1 + b]
    xc[:, :, 1:] = x_core.reshape(ROWS, NBLK, 128).transpose(2, 0, 1).astype(np_dt)
    return np.ascontiguousarray(xc)


def _unpack_core(yb: np.ndarray) -> np.ndarray:
    """[128, ROWS, NBLK] -> [ROWS, T] float32; yb[i, r, b] = y[r, 128 b + i]."""
    return yb.transpose(1, 2, 0).reshape(ROWS, T).astype(np.float32)


def kernel(x: np.ndarray, _trace: bool = False):
    from concourse.bass_utils import run_bass_kernel_spmd

    nc = _build_bass()

    x = np.asarray(x)
    assert x.shape == (B, T, 1), x.shape
    x2 = np.ascontiguousarray(x[:, :, 0], dtype=np.float32)

    in_maps = [
        {"xb": _pack_core(x2[c * ROWS : (c + 1) * ROWS])} for c in range(N_CORES)
    ]
    res = run_bass_kernel_spmd(nc, in_maps, list(range(N_CORES)), trace=_trace)

    y = np.empty((B, T), dtype=np.float32)
    for c in range(N_CORES):
        y[c * ROWS : (c + 1) * ROWS] = _unpack_core(res.results[c]["yb"])
    out = y[:, :, None]
    if _trace:
        return out, res
    return out


# revision 27
# speedup vs baseline: 1.0172x; 1.0172x over previous
"""Trainium2 kernel for nn_ButterworthFilter: 4th-order Butterworth lowpass
(scipy.signal.butter(4, 0.5) equivalent) applied as an IIR filter along time
for x of shape [256, 65536, 1], zero initial state per batch row.

Strategy
--------
The IIR impulse response decays below 1e-15 within 128 taps (max pole radius
0.7577), so the filter is numerically exactly a 128-tap causal FIR:

    y[t] = sum_{k=0}^{127} h[k] x[t-k]

Blocking time into 128-sample blocks, with X_cols[m, b] = x[128 b + m]:

    y[128 b + i] = sum_m W_A[m, i] X_cols[m, b] + sum_m W_B[m, i] X_cols[m, b-1]

with W_A[m, i] = h[i - m] (lower-triangular Toeplitz) and
W_B[m, i] = h[128 + i - m] (strictly upper-triangular). On the PE array this
is two accumulating matmuls per 512-block group with the contraction (m) on
partitions.

Sharding: pure data-parallel, 32 batch rows per core across 8 cores.

The natural->blocked layout change (and its inverse for y) is done on the
HOST, so the device only streams [128, 512] tiles: DMA in -> 2 matmuls ->
PSUM -> copy -> DMA out. The kernel is DMA-bound; mode "fp16" halves the
HBM traffic (absmax error ~5e-4 of output scale vs the fp32 reference;
mode "fp32r" keeps fp32 I/O at ~2.4e-4; mode "fp32" is ~1e-6 but slower).
"""
import os

import numpy as np

N_CORES = 8
B = 256
T = 65536
ROWS = B // N_CORES  # 32 batch rows per core
NBLK = T // 128  # 512 blocks of 128 samples per row
ORDER = 4

# per-chunk row counts: small chunks first (compute starts early) and last
# (short tail), big in the middle (few DMA triggers)
CHUNKS = [int(c) for c in os.environ.get("BUTTER_CHUNKS", "2,2,4,8,8,4,2,1,1").split(",")]
assert sum(CHUNKS) == ROWS
# "fp16" | "fp32r" | "fp32"
MODE = os.environ.get("BUTTER_MODE", "fp16")


def _design_fir(n_taps: int = 128) -> np.ndarray:
    """Butterworth(4, Wn=0.5) digital filter -> first n_taps of the impulse
    response, in float64. Same math as scipy.signal.butter(4, 0.5, 'low')."""
    fs2 = 4.0
    warped = fs2 * np.tan(np.pi * 0.5 / 4.0)
    k = np.arange(1, ORDER + 1)
    p = warped * np.exp(1j * np.pi * (2 * k + ORDER - 1) / (2 * ORDER))
    pd = (fs2 + p) / (fs2 - p)
    kd = (warped**ORDER) / np.real(np.prod(fs2 - p))
    b = np.real(kd * np.poly(-np.ones(ORDER)))
    a = np.real(np.poly(pd))

    h = np.zeros(n_taps)
    z = np.zeros(ORDER)
    for t in range(n_taps):
        xt = 1.0 if t == 0 else 0.0
        y = b[0] * xt + z[0]
        z = np.concatenate([z[1:], [0.0]]) + b[1:] * xt - a[1:] * y
        h[t] = y
    return h


def _toeplitz_weights() -> tuple[np.ndarray, np.ndarray]:
    h = _design_fir(128)
    idx = np.arange(128)
    d = idx[None, :] - idx[:, None]  # i - m
    w_a = np.where(d >= 0, h[np.clip(d, 0, 127)], 0.0)
    d2 = 128 + idx[None, :] - idx[:, None]
    w_b = np.where((d2 >= 1) & (d2 <= 127), h[np.clip(d2, 0, 127)], 0.0)
    return w_a.astype(np.float32), w_b.astype(np.float32)


_NC_CACHE = None

_IO_NP = {"fp16": np.float16, "fp32r": np.float32, "fp32": np.float32}


def _build_bass():
    """Build (and cache) the compiled per-core Bass program."""
    global _NC_CACHE
    if _NC_CACHE is not None:
        return _NC_CACHE

    import concourse.tile as tile
    from concourse import bacc, mybir

    w_a, w_b = _toeplitz_weights()

    if MODE == "fp16":
        io_dt = mm_dt = mybir.dt.float16
        w_a, w_b = w_a.astype(np.float16), w_b.astype(np.float16)
    elif MODE == "fp32r":
        io_dt = mm_dt = mybir.dt.float32r
    else:
        io_dt = mm_dt = mybir.dt.float32

    nc = bacc.Bacc("TRN2", target_bir_lowering=False, debug=False)
    # host-packed input, partition-major so each partition's DMA run is
    # CHUNK_ROWS*513 contiguous elements: [128, ROWS, 513];
    # [m, r, 0] = 0 (the b=-1 column), [m, r, 1 + b] = x[row r, 128 b + m]
    xb = nc.dram_tensor("xb", [128, ROWS, NBLK + 1], io_dt, kind="ExternalInput").ap()
    # output, partition-major: [128, ROWS, 512]; [i, r, b] = y[row r, 128 b + i]
    yb = nc.dram_tensor("yb", [128, ROWS, NBLK], io_dt, kind="ExternalOutput").ap()
    wa_dram = nc.inline_tensor(w_a, name="wa_const")
    wb_dram = nc.inline_tensor(w_b, name="wb_const")

    n_chunks = len(CHUNKS)

    with tile.TileContext(nc) as tc:
        with (
            tc.tile_pool(name="wpool", bufs=1) as wpool,
            tc.tile_pool(name="inp", bufs=1) as inp,
            tc.tile_pool(name="outp", bufs=1) as outp,
            tc.tile_pool(name="psum", bufs=8, space="PSUM") as psum_pool,
        ):
            wa_sb = wpool.tile([128, 128], mm_dt, tag="wa")
            nc.gpsimd.dma_start(wa_sb[:], wa_dram.ap().bitcast(mm_dt))
            wb_sb = wpool.tile([128, 128], mm_dt, tag="wb")
            nc.gpsimd.dma_start(wb_sb[:], wb_dram.ap().bitcast(mm_dt))

            half = NBLK // 2
            r0 = 0
            for c, crows in enumerate(CHUNKS):
                in_t = inp.tile([128, crows, NBLK + 1], io_dt, tag=f"in{c}")
                nc.sync.dma_start(in_t[:], xb[:, r0 : r0 + crows, :])
                out_t = outp.tile([128, crows, NBLK], io_dt, tag=f"out{c}")
                # group all W_A matmuls then all W_B matmuls so consecutive
                # matmuls share the stationary operand (amortize LDWEIGHTS)
                pss = []
                for r in range(crows):
                    ps = psum_pool.tile([128, NBLK], mybir.dt.float32, tag="ps")
                    pss.append(ps)
                    nc.tensor.matmul(
                        ps[:],
                        wa_sb[:],
                        in_t[:, r, 1 : NBLK + 1],
                        start=True,
                        stop=False,
                    )
                for r in range(crows):
                    nc.tensor.matmul(
                        pss[r][:],
                        wb_sb[:],
                        in_t[:, r, 0:NBLK],
                        start=False,
                        stop=True,
                    )
                for r in range(crows):
                    # split the PSUM->SBUF cast copy across DVE and ACT
                    nc.vector.tensor_copy(out_t[:, r, 0:half], pss[r][:, 0:half])
                    nc.scalar.copy(out_t[:, r, half:NBLK], pss[r][:, half:NBLK])
                nc.scalar.dma_start(yb[:, r0 : r0 + crows, :], out_t[:])
                r0 += crows

    nc.compile()
    _NC_CACHE = nc
    return nc


def _pack_core(x_core: np.ndarray) -> np.ndarray:
    """[ROWS, T] float32 -> [128, ROWS, NBLK+1] with a leading zero column."""
    np_dt = _IO_NP[MODE]
    xc = np.zeros((128, ROWS, NBLK + 1), dtype=np_dt)
    # x[row, 128 b + m] -> [m, row, 1 + b]
    xc[:, :, 1:] = x_core.reshape(ROWS, NBLK, 128).transpose(2, 0, 1).astype(np_dt)
    return np.ascontiguousarray(xc)


def _unpack_core(yb: np.ndarray) -> np.ndarray:
    """[128, ROWS, NBLK] -> [ROWS, T] float32; yb[i, r, b] = y[r, 128 b + i]."""
    return yb.transpose(1, 2, 0).reshape(ROWS, T).astype(np.float32)


def kernel(x: np.ndarray, _trace: bool = False):
    from concourse.bass_utils import run_bass_kernel_spmd

    nc = _build_bass()

    x = np.asarray(x)
    assert x.shape == (B, T, 1), x.shape
    x2 = np.ascontiguousarray(x[:, :, 0], dtype=np.float32)

    in_maps = [
        {"xb": _pack_core(x2[c * ROWS : (c + 1) * ROWS])} for c in range(N_CORES)
    ]
    res = run_bass_kernel_spmd(nc, in_maps, list(range(N_CORES)), trace=_trace)

    y = np.empty((B, T), dtype=np.float32)
    for c in range(N_CORES):
        y[c * ROWS : (c + 1) * ROWS] = _unpack_core(res.results[c]["yb"])
    out = y[:, :, None]
    if _trace:
        return out, res
    return out


# revision 28
# speedup vs baseline: 1.0809x; 1.0626x over previous
"""Trainium2 kernel for nn_ButterworthFilter: 4th-order Butterworth lowpass
(scipy.signal.butter(4, 0.5) equivalent) applied as an IIR filter along time
for x of shape [256, 65536, 1], zero initial state per batch row.

Strategy
--------
The IIR impulse response decays below 1e-15 within 128 taps (max pole radius
0.7577), so the filter is numerically exactly a 128-tap causal FIR:

    y[t] = sum_{k=0}^{127} h[k] x[t-k]

Blocking time into 128-sample blocks, with X_cols[m, b] = x[128 b + m]:

    y[128 b + i] = sum_m W_A[m, i] X_cols[m, b] + sum_m W_B[m, i] X_cols[m, b-1]

with W_A[m, i] = h[i - m] (lower-triangular Toeplitz) and
W_B[m, i] = h[128 + i - m] (strictly upper-triangular). On the PE array this
is two accumulating matmuls per 512-block group with the contraction (m) on
partitions.

Sharding: pure data-parallel, 32 batch rows per core across 8 cores.

The natural->blocked layout change (and its inverse for y) is done on the
HOST, so the device only streams [128, 512] tiles: DMA in -> 2 matmuls ->
PSUM -> copy -> DMA out. The kernel is DMA-bound; mode "fp16" halves the
HBM traffic (absmax error ~5e-4 of output scale vs the fp32 reference;
mode "fp32r" keeps fp32 I/O at ~2.4e-4; mode "fp32" is ~1e-6 but slower).
"""
import os

import numpy as np

N_CORES = 8
B = 256
T = 65536
ROWS = B // N_CORES  # 32 batch rows per core
NBLK = T // 128  # 512 blocks of 128 samples per row
ORDER = 4

# per-chunk row counts: small chunks first (compute starts early) and last
# (short tail), big in the middle (few DMA triggers)
CHUNKS = [int(c) for c in os.environ.get("BUTTER_CHUNKS", "2,2,4,8,8,4,2,1,1").split(",")]
assert sum(CHUNKS) == ROWS
# "fp16" | "fp32r" | "fp32"
MODE = os.environ.get("BUTTER_MODE", "fp16")


def _design_fir(n_taps: int = 128) -> np.ndarray:
    """Butterworth(4, Wn=0.5) digital filter -> first n_taps of the impulse
    response, in float64. Same math as scipy.signal.butter(4, 0.5, 'low')."""
    fs2 = 4.0
    warped = fs2 * np.tan(np.pi * 0.5 / 4.0)
    k = np.arange(1, ORDER + 1)
    p = warped * np.exp(1j * np.pi * (2 * k + ORDER - 1) / (2 * ORDER))
    pd = (fs2 + p) / (fs2 - p)
    kd = (warped**ORDER) / np.real(np.prod(fs2 - p))
    b = np.real(kd * np.poly(-np.ones(ORDER)))
    a = np.real(np.poly(pd))

    h = np.zeros(n_taps)
    z = np.zeros(ORDER)
    for t in range(n_taps):
        xt = 1.0 if t == 0 else 0.0
        y = b[0] * xt + z[0]
        z = np.concatenate([z[1:], [0.0]]) + b[1:] * xt - a[1:] * y
        h[t] = y
    return h


def _toeplitz_weights() -> tuple[np.ndarray, np.ndarray]:
    h = _design_fir(128)
    idx = np.arange(128)
    d = idx[None, :] - idx[:, None]  # i - m
    w_a = np.where(d >= 0, h[np.clip(d, 0, 127)], 0.0)
    d2 = 128 + idx[None, :] - idx[:, None]
    w_b = np.where((d2 >= 1) & (d2 <= 127), h[np.clip(d2, 0, 127)], 0.0)
    return w_a.astype(np.float32), w_b.astype(np.float32)


_NC_CACHE = None

_IO_NP = {"fp16": np.float16, "fp32r": np.float32, "fp32": np.float32}


def _build_bass():
    """Build (and cache) the compiled per-core Bass program."""
    global _NC_CACHE
    if _NC_CACHE is not None:
        return _NC_CACHE

    import concourse.tile as tile
    from concourse import bacc, mybir

    w_a, w_b = _toeplitz_weights()

    if MODE == "fp16":
        io_dt = mm_dt = mybir.dt.float16
        w_a, w_b = w_a.astype(np.float16), w_b.astype(np.float16)
    elif MODE == "fp32r":
        io_dt = mm_dt = mybir.dt.float32r
    else:
        io_dt = mm_dt = mybir.dt.float32

    nc = bacc.Bacc("TRN2", target_bir_lowering=False, debug=False)
    # host-packed input, partition-major so each partition's DMA run is
    # crows*513 contiguous elements: [128, ROWS, 513];
    # [m, r, 0] = 0 (the b=-1 column), [m, r, 1 + b] = x[row r, 128 b + m]
    xb = nc.dram_tensor("xb", [128, ROWS, NBLK + 1], io_dt, kind="ExternalInput").ap()
    # output, partition-major: [128, ROWS, 512]; [i, r, b] = y[row r, 128 b + i]
    yb = nc.dram_tensor("yb", [128, ROWS, NBLK], io_dt, kind="ExternalOutput").ap()
    wa_dram = nc.inline_tensor(w_a, name="wa_const")
    wb_dram = nc.inline_tensor(w_b, name="wb_const")

    n_chunks = len(CHUNKS)

    with tile.TileContext(nc) as tc:
        with (
            tc.tile_pool(name="wpool", bufs=1) as wpool,
            tc.tile_pool(name="inp", bufs=1) as inp,
            tc.tile_pool(name="outp", bufs=1) as outp,
            tc.tile_pool(name="psum", bufs=8, space="PSUM") as psum_pool,
        ):
            wa_sb = wpool.tile([128, 128], mm_dt, tag="wa")
            nc.gpsimd.dma_start(wa_sb[:], wa_dram.ap().bitcast(mm_dt))
            wb_sb = wpool.tile([128, 128], mm_dt, tag="wb")
            nc.gpsimd.dma_start(wb_sb[:], wb_dram.ap().bitcast(mm_dt))

            half = NBLK // 2
            r0 = 0
            for c, crows in enumerate(CHUNKS):
                in_t = inp.tile([128, crows, NBLK + 1], io_dt, tag=f"in{c}")
                nc.sync.dma_start(in_t[:], xb[:, r0 : r0 + crows, :])
                out_t = outp.tile([128, crows, NBLK], io_dt, tag=f"out{c}")
                # group all W_A matmuls then all W_B matmuls so consecutive
                # matmuls share the stationary operand (amortize LDWEIGHTS)
                pss = []
                for r in range(crows):
                    ps = psum_pool.tile([128, NBLK], mybir.dt.float32, tag="ps")
                    pss.append(ps)
                    nc.tensor.matmul(
                        ps[:],
                        wa_sb[:],
                        in_t[:, r, 1 : NBLK + 1],
                        start=True,
                        stop=False,
                    )
                for r in range(crows):
                    nc.tensor.matmul(
                        pss[r][:],
                        wb_sb[:],
                        in_t[:, r, 0:NBLK],
                        start=False,
                        stop=True,
                    )
                for r in range(crows):
                    # split the PSUM->SBUF cast copy across DVE and ACT
                    nc.vector.tensor_copy(out_t[:, r, 0:half], pss[r][:, 0:half])
                    nc.scalar.copy(out_t[:, r, half:NBLK], pss[r][:, half:NBLK])
                nc.scalar.dma_start(yb[:, r0 : r0 + crows, :], out_t[:])
                r0 += crows

    nc.compile()
    _NC_CACHE = nc
    return nc


def _pack_core(x_core: np.ndarray) -> np.ndarray:
    """[ROWS, T] float32 -> [128, ROWS, NBLK+1] with a leading zero column."""
    np_dt = _IO_NP[MODE]
    xc = np.zeros((128, ROWS, NBLK + 1), dtype=np_dt)
    # x[row, 128 b + m] -> [m, row, 1 + b]
    xc[:, :, 1:] = x_core.reshape(ROWS, NBLK, 128).transpose(2, 0, 1).astype(np_dt)
    return np.ascontiguousarray(xc)


def _unpack_core(yb: np.ndarray) -> np.ndarray:
    """[128, ROWS, NBLK] -> [ROWS, T] float32; yb[i, r, b] = y[r, 128 b + i]."""
    return yb.transpose(1, 2, 0).reshape(ROWS, T).astype(np.float32)


def kernel(x: np.ndarray, _trace: bool = False):
    from concourse.bass_utils import run_bass_kernel_spmd

    nc = _build_bass()

    x = np.asarray(x)
    assert x.shape == (B, T, 1), x.shape
    x2 = np.ascontiguousarray(x[:, :, 0], dtype=np.float32)

    in_maps = [
        {"xb": _pack_core(x2[c * ROWS : (c + 1) * ROWS])} for c in range(N_CORES)
    ]
    res = run_bass_kernel_spmd(nc, in_maps, list(range(N_CORES)), trace=_trace)

    y = np.empty((B, T), dtype=np.float32)
    for c in range(N_CORES):
        y[c * ROWS : (c + 1) * ROWS] = _unpack_core(res.results[c]["yb"])
    out = y[:, :, None]
    if _trace:
        return out, res
    return out
